# revision 8
# baseline (speedup 1.0000x reference)
"""Two-layer GCN (DGL GraphConv norm='both') on 8 Trainium2 NeuronCores.

Strategy (graph/data parallel, per sharding hint):
  - Nodes are range-partitioned across the 8 cores (1250 each); each core
    owns the dst-side segment_sum for its node range.
  - Each core receives ONE packed int16 input blob (~1 MB): its feature
    shard (norm_src pre-folded, bf16), a single per-edge src-index table
    (shared by both layers), per-window dst-slot ids, norm vectors, and
    the replicated weights.  A single packed input minimizes per-call
    PJRT/axon marshalling, which dominates single-shot latency.
  - The full (norm_src-scaled) feature table is assembled on-device via
    AllGather of the shards (the halo exchange); both layers' gathers
    read from on-device DRAM, never from host-replicated inputs.
  - Host pre-sorts each core's incoming edges by dst, groups them into
    128-row dst windows, and pads each window's edge list to a uniform
    block count so all 8 cores share one SPMD program.
  - Layer-1 aggregation: dma_gather pulls source feature rows from the
    AllGathered table; a per-block one-hot matrix M (built on-chip with
    iota + is_equal from precomputed local-dst ids) turns the segment sum
    into TensorEngine matmuls accumulating in PSUM:
        agg[dst,:] += M[edge,dst]^T @ Xg[edge,:]
  - H1 = (agg @ W1) * norm_dst + b1 (norm commutes through W1), ReLU, then
    z = (relu * norm_src) @ W2 is computed locally and AllGathered (bf16,
    padded to 256 cols so gather rows are 512B).  z rows live at their
    global node index (no per-core padding), so layer 2 reuses the SAME
    index table as layer 1.
  - Layer-2 aggregation: same gather + one-hot matmul scheme over z,
    then out = agg2 * norm_dst + b2.
"""

import numpy as np
import ml_dtypes

BF16 = ml_dtypes.bfloat16
N_CORES = 8

LAST_STATS = {}


def _pad128(n):
    return (n + 127) // 128 * 128


def _offsets(N, F, H, C, NT, B):
    """Packed-blob int16-element offsets; derived only from shapes so the
    builder does not need host data."""
    npc = N // N_CORES
    NE = NT * B * 128
    NBLK = NT * B
    offs = {}
    o = 0
    for name, sz in [
        ("x", npc * F),            # bf16 feature shard
        ("idx", NE),               # int16 src indices [16, NE/16]
        ("dl", 128 * NBLK),        # bf16 dst-slot ids [128, NBLK]
        ("nd", 128 * NT * 2),      # f32 norm_dst [128, NT]
        ("ns", 128 * NT * 2),      # f32 norm_src(own) [128, NT]
        ("w1", F * H),             # bf16
        ("w2", H * C),             # bf16
        ("b1", 128 * 2 * H),       # f32 [128, H] (host-replicated)
        ("b2", 128 * 2 * C),       # f32 [128, C] (host-replicated)
    ]:
        offs[name] = o
        o += _pad128(sz)
    return offs, o


def _prep(features, W1, b1, W2, b2, src, dst):
    N, F = features.shape
    H = W1.shape[1]
    C = W2.shape[1]
    assert N % N_CORES == 0
    npc = N // N_CORES            # nodes per core
    NT = (npc + 127) // 128       # dst windows per core

    src = np.asarray(src, np.int64)
    dst = np.asarray(dst, np.int64)

    out_deg = np.bincount(src, minlength=N).astype(np.float32)
    in_deg = np.bincount(dst, minlength=N).astype(np.float32)
    norm_src = (1.0 / np.sqrt(np.clip(out_deg, 1.0, None))).astype(np.float32)
    norm_dst = (1.0 / np.sqrt(np.clip(in_deg, 1.0, None))).astype(np.float32)

    # norm_src folded into the feature shards (the table is AllGathered
    # on-device and gathered per edge from there)
    featb = (np.asarray(features, np.float32) * norm_src[:, None]).astype(BF16)

    order = np.argsort(dst, kind="stable")
    ds = dst[order]
    ss = src[order]

    # per (core, window) counts
    i0 = np.empty((N_CORES, NT), np.int64)
    i1 = np.empty((N_CORES, NT), np.int64)
    for c in range(N_CORES):
        for w in range(NT):
            lo = c * npc + w * 128
            hi = min(lo + 128, (c + 1) * npc)
            i0[c, w] = np.searchsorted(ds, lo, "left")
            i1[c, w] = np.searchsorted(ds, hi, "left")
    counts = i1 - i0
    B = int(np.ceil(counts.max() / 128))      # blocks per window (uniform)
    EW = B * 128                              # padded edges per window
    NE = NT * EW                              # padded edges per core
    NBLK = NT * B

    offs, TOT = _offsets(N, F, H, C, NT, B)
    W1b = np.asarray(W1, np.float32).astype(BF16)
    W2b = np.asarray(W2, np.float32).astype(BF16)
    b1f = np.broadcast_to(np.asarray(b1, np.float32).reshape(1, H),
                          (128, H)).copy()
    b2f = np.broadcast_to(np.asarray(b2, np.float32).reshape(1, C),
                          (128, C)).copy()

    blobs = []
    for c in range(N_CORES):
        s1 = np.zeros(NE, np.int64)
        dl = np.full(NE, -1.0, np.float32)
        for w in range(NT):
            a, b = i0[c, w], i1[c, w]
            cnt = b - a
            pos = w * EW
            s1[pos:pos + cnt] = ss[a:b]
            dl[pos:pos + cnt] = (ds[a:b] - (c * npc + w * 128)).astype(np.float32)
        idx16 = s1.reshape(NE // 16, 16).T.astype(np.int16)   # [16, NE/16]
        dlw = dl.reshape(NBLK, 128).T.astype(BF16)            # [128, NBLK]

        npc_pad = NT * 128
        own_nd = np.ones(npc_pad, np.float32)
        own_nd[:npc] = norm_dst[c * npc:(c + 1) * npc]
        own_ns = np.ones(npc_pad, np.float32)
        own_ns[:npc] = norm_src[c * npc:(c + 1) * npc]

        blob = np.zeros(TOT, np.int16)

        def put(name, arr):
            a16 = np.ascontiguousarray(arr).view(np.int16).ravel()
            blob[offs[name]:offs[name] + a16.size] = a16

        put("x", featb[c * npc:(c + 1) * npc])
        put("idx", idx16)
        put("dl", dlw)
        put("nd", own_nd.reshape(NT, 128).T)
        put("ns", own_ns.reshape(NT, 128).T)
        put("w1", W1b)
        put("w2", W2b)
        put("b1", b1f)
        put("b2", b2f)
        blobs.append(blob)

    shapes = dict(N=N, F=F, H=H, C=C, NT=NT, B=B, NE=NE, npc=npc)
    in_maps = [dict(blob=blobs[c]) for c in range(N_CORES)]
    return shapes, in_maps


def _build(N, F, H, C, NT, B, NE, variant="full", num_devices=N_CORES,
           repeat=1):
    import concourse.bacc as bacc
    import concourse.mybir as mybir
    from concourse import tile

    dt = mybir.dt
    alu = mybir.AluOpType
    KF = F // 128   # feat chunks
    KH = H // 128   # hidden chunks
    npc = N // N_CORES
    NBLK = NT * B
    offs, TOT = _offsets(N, F, H, C, NT, B)

    nc = bacc.Bacc("TRN2", target_bir_lowering=False, debug=False,
                   num_devices=num_devices, num_swdge_queues=4)

    blob_d = nc.dram_tensor("blob", [TOT], dt.int16, kind="ExternalInput")
    out_d = nc.dram_tensor("out", [NT * 128, C], dt.float32,
                           kind="ExternalOutput")

    def bap(name, n):
        o = offs[name]
        return blob_d.ap()[o:o + n]

    with tile.TileContext(nc) as tc:
        with (
            tc.tile_pool(name="const", bufs=1) as const,
            tc.tile_pool(name="dram", bufs=1, space="DRAM") as dram,
            tc.tile_pool(name="zf", bufs=repeat, space="DRAM") as zf_pool,
            tc.tile_pool(name="xg", bufs=3) as xg_pool,
            tc.tile_pool(name="work", bufs=3) as work,
            tc.tile_pool(name="ps_agg", bufs=2, space="PSUM") as ps_agg,
            tc.tile_pool(name="ps_tr", bufs=2, space="PSUM") as ps_tr,
            tc.tile_pool(name="ps_h", bufs=2, space="PSUM") as ps_h,
        ):
            # ---- constants (all DMA'd out of the single packed blob) ----
            idx_t = const.tile([128, NE // 16], dt.int16)
            idx_src = bap("idx", NE).rearrange("(p t) -> p t", t=NE // 16)
            for q in range(8):   # replicate across the 8 Q7 16-partition slices
                nc.sync.dma_start(idx_t[16 * q:16 * q + 16, :], idx_src)
            dlbf_t = const.tile([128, NBLK], dt.bfloat16)
            nc.sync.dma_start(
                dlbf_t[:],
                bap("dl", 128 * NBLK).rearrange("(p t) -> p t", t=NBLK)
                .bitcast(dt.bfloat16))
            ndst_t = const.tile([128, NT], dt.float32)
            nc.sync.dma_start(
                ndst_t[:],
                bap("nd", 128 * NT * 2).rearrange("(p t) -> p t", t=2 * NT)
                .bitcast(dt.float32))
            nso_t = const.tile([128, NT], dt.float32)
            nc.sync.dma_start(
                nso_t[:],
                bap("ns", 128 * NT * 2).rearrange("(p t) -> p t", t=2 * NT)
                .bitcast(dt.float32))

            w1_sb = const.tile([128, KF, H], dt.bfloat16)
            for k in range(KF):
                nc.sync.dma_start(
                    w1_sb[:, k, :],
                    bap("w1", F * H)[k * 128 * H:(k + 1) * 128 * H]
                    .rearrange("(p n) -> p n", n=H).bitcast(dt.bfloat16))
            w2_sb = const.tile([128, KH, C], dt.bfloat16)
            for k in range(KH):
                nc.sync.dma_start(
                    w2_sb[:, k, :],
                    bap("w2", H * C)[k * 128 * C:(k + 1) * 128 * C]
                    .rearrange("(p n) -> p n", n=C).bitcast(dt.bfloat16))
            b1_bc = const.tile([128, H], dt.float32)
            nc.sync.dma_start(
                b1_bc[:],
                bap("b1", 128 * 2 * H).rearrange("(p n) -> p n", n=2 * H)
                .bitcast(dt.float32))
            b2_bc = const.tile([128, C], dt.float32)
            nc.sync.dma_start(
                b2_bc[:],
                bap("b2", 128 * 2 * C).rearrange("(p n) -> p n", n=2 * C)
                .bitcast(dt.float32))

            iota_bf = const.tile([128, B, 128], dt.bfloat16)
            nc.gpsimd.iota(iota_bf[:], pattern=[[0, B], [1, 128]], base=0,
                           channel_multiplier=0,
                           allow_small_or_imprecise_dtypes=True)
            m_all = const.tile([128, NBLK, 128], dt.bfloat16)
            nc.vector.tensor_tensor(
                m_all[:], iota_bf[:, 0:1, :].broadcast_to((128, NBLK, 128)),
                dlbf_t[:, :].broadcast_to((128, NBLK, 128)),
                alu.is_equal)
            iota_col = const.tile([128, 1], dt.float32)
            nc.gpsimd.iota(iota_col[:], pattern=[[0, 1]], base=0,
                           channel_multiplier=1,
                           allow_small_or_imprecise_dtypes=True)
            ident_bf = const.tile([128, 128], dt.bfloat16)
            nc.vector.tensor_scalar(ident_bf[:], iota_bf[:, 0, :], iota_col[:],
                                    None, alu.is_equal)


            # feature shard -> internal DRAM, then AllGather = halo exchange
            xin = dram.tile([npc, F], dt.bfloat16)
            nc.sync.dma_start(
                xin[:, :],
                bap("x", npc * F).rearrange("(r c) -> r c", c=F)
                .bitcast(dt.bfloat16))
            cc_in = dram.tile([npc, 128], dt.bfloat16)

            for _rep in range(repeat):
                featb_full = zf_pool.tile([N_CORES * npc, F], dt.bfloat16,
                                          addr_space="Shared", tag="xf")
                if variant == "no_xag" or num_devices == 1:
                    nc.sync.dma_start(featb_full[0:npc, :], xin[:, :])
                else:
                    nc.gpsimd.collective_compute(
                        "AllGather", alu.bypass,
                        replica_groups=[list(range(N_CORES))],
                        ins=[xin.opt()], outs=[featb_full.opt()])
                z_full = zf_pool.tile([N_CORES * npc, 128], dt.bfloat16,
                                      addr_space="Shared", tag="zf")
                zbuf = work.tile([128, NT, 128], dt.bfloat16, tag="zbuf")
                nc.vector.memset(zbuf[:], 0.0)
                obuf = work.tile([128, NT, C], dt.float32, tag="obuf")

                # ---- layer 1 ----
                for w in range(NT):
                    xg = xg_pool.tile([128, B, F], dt.bfloat16, tag="xg")
                    if variant == "no_gather":
                        nc.vector.memset(xg[:], 0.25)
                    elif variant == "no_l1gather":
                        nc.vector.memset(xg[:, 0, 0:16], 0.25)
                    else:
                        nc.gpsimd.dma_gather(
                            xg[:, :, :], featb_full[:, :],
                            idx_t[:, w * B * 8:(w + 1) * B * 8],
                            B * 128, B * 128, F,
                            single_packet=(B * 128 <= 1024),
                            queue_num=w % 4)
                    agg = ps_agg.tile([128, F], dt.float32, tag="agg")
                    for b in range(B):
                        nc.tensor.matmul(agg[:], lhsT=m_all[:, w * B + b, :],
                                         rhs=xg[:, b, :],
                                         start=(b == 0), stop=(b == B - 1))
                    aggc = work.tile([128, F], dt.bfloat16, tag="aggc")
                    nc.vector.tensor_copy(aggc[:], agg[:])
                    aggT = work.tile([128, KF, 128], dt.bfloat16, tag="aggT")
                    for k in range(KF):
                        trp = ps_tr.tile([128, 128], dt.bfloat16, tag="tr")
                        nc.tensor.transpose(trp[:], aggc[:, k * 128:(k + 1) * 128],
                                            ident_bf[:])
                        nc.vector.tensor_copy(aggT[:, k, :], trp[:])
                    h1 = ps_h.tile([128, H], dt.float32, tag="h")
                    for k in range(KF):
                        nc.tensor.matmul(h1[:], lhsT=aggT[:, k, :], rhs=w1_sb[:, k, :],
                                         start=(k == 0), stop=(k == KF - 1))
                    t1 = work.tile([128, H], dt.float32, tag="t1")
                    nc.vector.scalar_tensor_tensor(t1[:], h1[:], ndst_t[:, w:w + 1],
                                                   b1_bc[:], alu.mult, alu.add)
                    yz = work.tile([128, H], dt.bfloat16, tag="yz")
                    nc.scalar.activation(yz[:], t1[:],
                                         mybir.ActivationFunctionType.Relu,
                                         scale=nso_t[:, w:w + 1])
                    yzT = work.tile([128, KH, 128], dt.bfloat16, tag="yzT")
                    for k in range(KH):
                        trp2 = ps_tr.tile([128, 128], dt.bfloat16, tag="tr")
                        nc.tensor.transpose(trp2[:], yz[:, k * 128:(k + 1) * 128],
                                            ident_bf[:])
                        nc.vector.tensor_copy(yzT[:, k, :], trp2[:])
                    zn = ps_h.tile([128, C], dt.float32, tag="zn")
                    for k in range(KH):
                        nc.tensor.matmul(zn[:], lhsT=yzT[:, k, :], rhs=w2_sb[:, k, :],
                                         start=(k == 0), stop=(k == KH - 1))
                    nc.vector.tensor_copy(zbuf[:, w, 0:C], zn[:])

                # flush z windows to DRAM (node-major rows; npc=1250 is
                # not window-aligned, so the tail window goes separately)
                WF = npc // 128
                nc.sync.dma_start(
                    cc_in[0:WF * 128, :].rearrange("(w p) c -> p w c", p=128),
                    zbuf[:, 0:WF, :])
                nc.sync.dma_start(cc_in[WF * 128:npc, :],
                                  zbuf[0:npc - WF * 128, WF, :])

                # ---- halo exchange of z (rows at global node index) ----
                if variant == "no_cc" or num_devices == 1:
                    nc.sync.dma_start(z_full[0:npc, :], cc_in[:, :])
                else:
                    nc.gpsimd.collective_compute(
                        "AllGather", alu.bypass,
                        replica_groups=[list(range(N_CORES))],
                        ins=[cc_in.opt()], outs=[z_full.opt()])

                # ---- layer 2 ----
                for w in range(NT):
                    # z rows are 256B (the dma_gather minimum read); cols
                    # C..128 are pad, dropped by the matmul rhs slice.
                    xg2 = xg_pool.tile([128, B, 128], dt.bfloat16, tag="xg2")
                    if variant == "no_l2gather_pure":
                        nc.vector.memset(xg2[:, 0, 0:16], 0.25)
                    elif variant in ("no_gather", "no_l2gather"):
                        nc.vector.memset(xg2[:], 0.25)
                    else:
                        nc.gpsimd.dma_gather(
                            xg2[:, :, :], z_full[:, :],
                            idx_t[:, w * B * 8:(w + 1) * B * 8],
                            B * 128, B * 128, 128,
                            single_packet=(B * 128 <= 1024),
                            queue_num=w % 4)
                    agg2 = ps_agg.tile([128, C], dt.float32, tag="agg")
                    for b in range(B):
                        nc.tensor.matmul(agg2[:], lhsT=m_all[:, w * B + b, :],
                                         rhs=xg2[:, b, 0:C],
                                         start=(b == 0), stop=(b == B - 1))
                    nc.vector.scalar_tensor_tensor(obuf[:, w, :], agg2[:],
                                                   ndst_t[:, w:w + 1], b2_bc[:],
                                                   alu.mult, alu.add)
                if variant != "no_out":
                    nc.sync.dma_start(
                        out_d.ap().rearrange("(w p) c -> p w c", p=128),
                        obuf[:, :, :])

    nc.compile()
    return nc


def kernel(features, W1, b1, W2, b2, src, dst, **_):
    import time
    from concourse.bass_utils import run_bass_kernel_spmd

    t0 = time.time()
    shapes, in_maps = _prep(features, W1, b1, W2, b2, src, dst)
    t1 = time.time()
    nc = _build(shapes["N"], shapes["F"], shapes["H"], shapes["C"],
                shapes["NT"], shapes["B"], shapes["NE"])
    t2 = time.time()
    res = run_bass_kernel_spmd(nc, in_maps, core_ids=list(range(N_CORES)))
    t3 = time.time()
    npc = shapes["npc"]
    out = np.concatenate([res.results[c]["out"][:npc] for c in range(N_CORES)], 0)
    LAST_STATS.update(prep_s=t1 - t0, build_s=t2 - t1, run_s=t3 - t2,
                      B=shapes["B"], NE=shapes["NE"])
    return np.ascontiguousarray(out.astype(np.float32))
